# revision 8
# baseline (speedup 1.0000x reference)
"""Trainium2 Bass kernel for nn_BasicLSTM (B=64, T=512, D=512, U=1024).

Strategy: data-parallel over batch across 8 NeuronCores (8 sequences per
core, recurrence fully local per core — no cross-core communication).

Per-core step t computes z = [x_t, h, 1] @ W  as 13 K-chunks x 8 N-tiles of
512 columns.  N-tiles are spread over the PE array's four 32-row column
groups via tile_position, so four matmuls stream concurrently; the 8-row
(batch) outputs land at PSUM partition offsets {0,32,64,96}.  Weight columns
are host-permuted to [i|f|o|g] per 512-unit bank with the g-gate columns
pre-scaled by 2 so one sigmoid pass per bank covers every gate
(tanh(x) = 2*sigmoid(2x) - 1).  Weights, x (transposed host-side) and all
state live in SBUF for the whole kernel: the only DMAs are the initial
loads and the final store.  Matmul operands are bf16 (fp32 PSUM
accumulation); the cell state c stays fp32.
"""

import numpy as np
import ml_dtypes

B, T, D, U = 64, 512, 512, 1024
NCORES = 8
BL = B // NCORES          # 8 sequences per core
NK_X = D // 128           # 4 x K-chunks
NK_H = U // 128           # 8 h K-chunks
NT = 512                  # N-tile width (one PSUM bank)
GATE_OFF = (0, 32, 64, 96)  # PSUM partition offset per col-group (i,f,o,g)


def _build_nc(t_steps=T):
    import concourse.bass as bass
    import concourse.mybir as mybir

    f32, bf16 = mybir.dt.float32, mybir.dt.bfloat16
    AF = mybir.ActivationFunctionType
    ALU = mybir.AluOpType

    nc = bass.Bass(num_devices=NCORES)
    wq = nc.declare_dram_parameter("wq", [1537, 4096], bf16, isOutput=False)
    xq = nc.declare_dram_parameter("xq", [NK_X, 128, t_steps, BL], bf16, isOutput=False)
    ib_d = nc.declare_dram_parameter("ib", [BL + 1, BL], bf16, isOutput=False)
    zz_d = nc.declare_dram_parameter("zz", [128, NT], bf16, isOutput=False)
    cz_d = nc.declare_dram_parameter("cz", [BL, U], f32, isOutput=False)
    out_d = nc.declare_dram_parameter("out", [BL, U], f32, isOutput=True)

    from contextlib import ExitStack
    ctx = ExitStack()
    sb = lambda shape, dt, name: ctx.enter_context(nc.sbuf_tensor(name, shape, dt))
    ps = lambda shape, dt, name: ctx.enter_context(nc.psum_tensor(name, shape, dt))
    sem = lambda name: ctx.enter_context(nc.semaphore(name))

    with ctx:
        w_sb = sb([128, 12 * 4096], bf16, "w_sb")
        bias_sb = sb([1, 4096], bf16, "bias_sb")
        x_sb = sb([128, NK_X * t_steps * BL], bf16, "x_sb")
        ones_sb = sb([1, BL], bf16, "ones_sb")
        ident = sb([BL, BL], bf16, "ident")
        s_sb = [sb([128, NT], bf16, f"s_sb{b}") for b in range(2)]
        g2_sb = [sb([BL, NT], bf16, f"g2_sb{b}") for b in range(2)]
        # operands of 2-input DVE ops must share a base partition; slice at
        # the offsets where the sigmoid output lives (f at 32, o at 64)
        t1_sb = [sb([32 + BL, NT], f32, f"t1_sb{b}") for b in range(2)]
        c1_sb = [sb([32 + BL, NT], f32, f"c1_sb{b}") for b in range(2)]
        tc_sb = [sb([64 + BL, NT], bf16, f"tc_sb{b}") for b in range(2)]
        h_sb = [sb([BL, NT], bf16, f"h_sb{b}") for b in range(2)]
        c_sb = sb([32 + BL, U], f32, "c_sb")
        hT_sb = sb([128, 2 * NK_H * BL], bf16, "hT_sb")  # double buffered h.T
        hlast_sb = sb([BL, U], f32, "hlast_sb")
        zeros_sb = sb([128, NT], bf16, "zeros_sb")

        zbuf = [ps([128, NT], f32, f"zbuf{i}") for i in range(4)]
        trbuf = [ps([128, 2 * NK_H * BL // 2], bf16, f"trbuf{i}") for i in range(2)]
        # trbuf: [128, 64] bf16; bank A transposes cols 0:32, bank B 32:64

        dma_sem = sem("dma_sem")
        mm_sem = sem("mm_sem")
        sig_sem = sem("sig_sem")
        csum_sem = sem("csum_sem")
        tanh_sem = sem("tanh_sem")
        h_sem = sem("h_sem")
        tr_sem = sem("tr_sem")
        cp_sem = sem("cp_sem")
        clr_sem = sem("clr_sem")

        with nc.Block() as block:

            @block.sync
            def _(sync):
                n = 0
                for kc in range(12):
                    sync.dma_start(
                        out=w_sb[:, kc * 4096:(kc + 1) * 4096],
                        in_=wq[kc * 128:(kc + 1) * 128, :],
                    ).then_inc(dma_sem, 16)
                    n += 1
                sync.dma_start(out=bias_sb[:, :], in_=wq[1536:1537, :]).then_inc(dma_sem, 16)
                n += 1
                for kc in range(NK_X):
                    sync.dma_start(
                        out=x_sb[:, kc * (t_steps * BL):(kc + 1) * (t_steps * BL)],
                        in_=xq[kc],
                    ).then_inc(dma_sem, 16)
                    n += 1
                sync.dma_start(out=ident[:, :], in_=ib_d[0:BL, :]).then_inc(dma_sem, 16)
                sync.dma_start(out=ones_sb[:, :], in_=ib_d[BL:BL + 1, :]).then_inc(dma_sem, 16)
                sync.dma_start(out=c_sb[32:32 + BL, :], in_=cz_d[:, :]).then_inc(dma_sem, 16)
                sync.dma_start(out=zeros_sb[:, :], in_=zz_d[:, :]).then_inc(dma_sem, 16)
                n += 4
                # final store
                sync.wait_ge(h_sem, 2 * t_steps)
                sync.dma_start(out=out_d[:, :], in_=hlast_sb[:, :]).then_inc(dma_sem, 16)

            @block.tensor
            def _(tensor):
                tensor.wait_ge(dma_sem, 16 * (12 + 1 + NK_X + 4))
                for t in range(t_steps):
                    zA = zbuf[(t % 2) * 2]
                    zB = zbuf[(t % 2) * 2 + 1]
                    rd_buf = (t + 1) % 2   # h.T written at t-1
                    # banks for this step must be zeroed (init 4, then 2/step)
                    tensor.wait_ge(clr_sem, 4 + 2 * max(0, t - 1))
                    # x + bias matmuls (no dependence on h)
                    for bk, z in ((0, zA), (1, zB)):
                        for kc in range(NK_X):
                            lhsT = x_sb[:, kc * (t_steps * BL) + t * BL:
                                        kc * (t_steps * BL) + (t + 1) * BL]
                            for cg in range(4):
                                ncol = (4 * bk + cg) * NT
                                tensor.matmul(
                                    z[GATE_OFF[cg]:GATE_OFF[cg] + BL, :],
                                    lhsT,
                                    w_sb[:, kc * 4096 + ncol:kc * 4096 + ncol + NT],
                                    start=False,
                                    stop=False,
                                    tile_position=(0, GATE_OFF[cg]),
                                    skip_group_check=True,
                                )
                        # bias row (K=1); at t=0 this is also the last matmul
                        # of the accumulation group (h=0 -> no h terms)
                        for cg in range(4):
                            ncol = (4 * bk + cg) * NT
                            last = (t == 0 and cg == 3)
                            ins = tensor.matmul(
                                z[GATE_OFF[cg]:GATE_OFF[cg] + BL, :],
                                ones_sb[0:1, :],
                                bias_sb[0:1, ncol:ncol + NT],
                                start=False,
                                stop=last,
                                tile_position=(0, GATE_OFF[cg]),
                                skip_group_check=True,
                            )
                            if last:
                                ins.then_inc(mm_sem, 1)
                    # h matmuls (skip at t=0: h=0)
                    if t > 0:
                        tensor.wait_ge(cp_sem, 2 * t)
                        for bk, z in ((0, zA), (1, zB)):
                            for j in range(NK_H):
                                kc = NK_X + j
                                lhsT = hT_sb[:, rd_buf * (NK_H * BL) + j * BL:
                                             rd_buf * (NK_H * BL) + (j + 1) * BL]
                                for cg in range(4):
                                    ncol = (4 * bk + cg) * NT
                                    last = (j == NK_H - 1 and cg == 3)
                                    ins = tensor.matmul(
                                        z[GATE_OFF[cg]:GATE_OFF[cg] + BL, :],
                                        lhsT,
                                        w_sb[:, kc * 4096 + ncol:kc * 4096 + ncol + NT],
                                        start=False,
                                        stop=last,
                                        tile_position=(0, GATE_OFF[cg]),
                                        skip_group_check=True,
                                    )
                                    if last:
                                        ins.then_inc(mm_sem, 1)
                    # transposes of h (not needed after final step)
                    if t < t_steps - 1:
                        for bk in range(2):
                            tensor.wait_ge(h_sem, 2 * t + bk + 1)
                            for j in range(4):
                                ins = tensor.matmul(
                                    trbuf[t % 2][:, (bk * 4 + j) * BL:(bk * 4 + j + 1) * BL],
                                    h_sb[bk][0:BL, j * 128:(j + 1) * 128],
                                    ident[:, :],
                                    start=True,
                                    stop=True,
                                    is_transpose=True,
                                    skip_group_check=True,
                                )
                                if j == 3:
                                    ins.then_inc(tr_sem, 1)

            @block.scalar
            def _(scalar):
                scalar.wait_ge(dma_sem, 16 * (12 + 1 + NK_X + 4))
                for i_ in range(4):
                    nc.scalar.copy(zbuf[i_][:, :], zeros_sb[:, :]).then_inc(clr_sem, 1)
                for t in range(t_steps):
                    zA = zbuf[(t % 2) * 2]
                    zB = zbuf[(t % 2) * 2 + 1]
                    for bk, z in ((0, zA), (1, zB)):
                        scalar.wait_ge(mm_sem, 2 * t + bk + 1)
                        nc.scalar.activation(
                            s_sb[bk][:, :], z[:, :], mybir.ActivationFunctionType.Sigmoid,
                        ).then_inc(sig_sem, 1)
                    for bk in range(2):
                        scalar.wait_ge(csum_sem, 2 * t + bk + 1)
                        nc.scalar.activation(
                            tc_sb[bk][64:64 + BL, :], c_sb[32:32 + BL, bk * NT:(bk + 1) * NT],
                            mybir.ActivationFunctionType.Tanh,
                        ).then_inc(tanh_sem, 1)
                    if t < t_steps - 1:
                        for bk in range(2):
                            scalar.wait_ge(tr_sem, 2 * t + bk + 1)
                            nc.scalar.copy(
                                hT_sb[:, (t % 2) * (NK_H * BL) + bk * 4 * BL:
                                      (t % 2) * (NK_H * BL) + (bk + 1) * 4 * BL],
                                trbuf[t % 2][:, bk * 4 * BL:(bk + 1) * 4 * BL],
                            ).then_inc(cp_sem, 1)
                    if t < t_steps - 2:
                        # re-zero this step's banks for reuse at t+2
                        nc.scalar.copy(zbuf[(t % 2) * 2][:, :], zeros_sb[:, :]).then_inc(clr_sem, 1)
                        nc.scalar.copy(zbuf[(t % 2) * 2 + 1][:, :], zeros_sb[:, :]).then_inc(clr_sem, 1)

            @block.vector
            def _(vector):
                ALU = mybir.AluOpType
                for t in range(t_steps):
                    # drain orders this step's reads after last step's writes
                    vector.drain()
                    for bk in range(2):
                        s = s_sb[bk]
                        vector.wait_ge(sig_sem, 2 * t + bk + 1)
                        nc.vector.tensor_scalar(
                            g2_sb[bk][:, :], s[96:96 + BL, :], 2.0, -1.0,
                            ALU.mult, ALU.add,
                        )
                        nc.vector.tensor_mul(
                            c1_sb[bk][32:32 + BL, :], s[32:32 + BL, :],
                            c_sb[32:32 + BL, bk * NT:(bk + 1) * NT],
                        )
                    vector.drain()
                    for bk in range(2):
                        nc.vector.tensor_mul(
                            t1_sb[bk][32:32 + BL, :], s_sb[bk][0:BL, :], g2_sb[bk][:, :])
                    vector.drain()
                    for bk in range(2):
                        nc.vector.tensor_add(
                            c_sb[32:32 + BL, bk * NT:(bk + 1) * NT],
                            c1_sb[bk][32:32 + BL, :], t1_sb[bk][32:32 + BL, :],
                        ).then_inc(csum_sem, 1)
                    for bk in range(2):
                        vector.wait_ge(tanh_sem, 2 * t + bk + 1)
                        if t < t_steps - 1:
                            nc.vector.tensor_mul(
                                h_sb[bk][:, :], s_sb[bk][64:64 + BL, :],
                                tc_sb[bk][64:64 + BL, :],
                            ).then_inc(h_sem, 1)
                        else:
                            nc.vector.tensor_mul(
                                hlast_sb[0:BL, bk * NT:(bk + 1) * NT],
                                s_sb[bk][64:64 + BL, :],
                                tc_sb[bk][64:64 + BL, :],
                            ).then_inc(h_sem, 1)

    return nc


def _prep_inputs(x, Wx, Wh, b):
    """Host-side layout prep (pure layout/dtype, no compute)."""
    bf16 = ml_dtypes.bfloat16
    t_steps = x.shape[1]
    # W = [Wx; Wh; b] rows, columns permuted to per-bank [i|f|o|g] blocks.
    Wfull = np.concatenate([Wx, Wh, b[None, :]], axis=0).astype(np.float32)
    # original gate column ranges: i 0:U, f U:2U, g 2U:3U, o 3U:4U
    cols = []
    for bank in range(2):
        u0, u1 = bank * NT, (bank + 1) * NT
        cols.append(np.arange(0 * U + u0, 0 * U + u1))       # i
        cols.append(np.arange(1 * U + u0, 1 * U + u1))       # f
        cols.append(np.arange(3 * U + u0, 3 * U + u1))       # o
        cols.append(np.arange(2 * U + u0, 2 * U + u1))       # g
    perm = np.concatenate(cols)
    Wp = Wfull[:, perm].copy()
    # pre-scale g-gate columns by 2 (tanh(x) = 2*sigmoid(2x)-1)
    for bank in range(2):
        g0 = bank * 4 * NT + 3 * NT
        Wp[:, g0:g0 + NT] *= 2.0
    Wp = np.ascontiguousarray(Wp).astype(bf16)

    # per-core x, transposed: xq[kc, p, t, b] = x[core*BL+b, t, kc*128+p]
    xqs = []
    for core in range(NCORES):
        xs = x[core * BL:(core + 1) * BL].astype(np.float32)      # [BL, T, D]
        xt = np.ascontiguousarray(np.transpose(xs, (2, 1, 0)))    # [D, T, BL]
        xt = xt.reshape(NK_X, 128, t_steps, BL)
        xqs.append(np.ascontiguousarray(xt).astype(bf16))
    return Wp, xqs


def kernel(x, Wx, Wh, b):
    x = np.asarray(x, dtype=np.float32)
    Wx = np.asarray(Wx, dtype=np.float32)
    Wh = np.asarray(Wh, dtype=np.float32)
    b = np.asarray(b, dtype=np.float32)
    t_steps = x.shape[1]

    Wp, xqs = _prep_inputs(x, Wx, Wh, b)
    nc = _build_nc(t_steps)

    from concourse.bass_utils import run_bass_kernel_spmd
    core_ids = list(range(NCORES))
    ib = np.zeros((BL + 1, BL), dtype=ml_dtypes.bfloat16)
    for i_ in range(BL):
        ib[i_, i_] = 1.0
    ib[BL, :] = 1.0
    cz = np.zeros((BL, U), dtype=np.float32)
    zz = np.zeros((128, NT), dtype=ml_dtypes.bfloat16)
    in_maps = [{"wq": Wp, "xq": xqs[i], "ib": ib, "cz": cz, "zz": zz} for i in core_ids]
    res = run_bass_kernel_spmd(nc, in_maps, core_ids, trace=bool(globals().get("TRACE", False)))
    globals()["LAST_EXEC_NS"] = res.exec_time_ns

    # unshard: bank A = units 0:512, bank B = 512:1024 (identity unit order)
    h_parts = [res.results[i]["out"].astype(np.float32) for i in core_ids]
    return np.concatenate(h_parts, axis=0)
